# revision 21
# baseline (speedup 1.0000x reference)
"""Trainium2 Bass kernel for batched masked softmax attention.

Problem: B=16, S=2048, D=1024 fp32
    e = (Q @ K^T) / sqrt(D)
    e -= (1 - mask[:, :, None]) * 1e9        # per query-row constant
    a = softmax(e, axis=-1)
    out = a @ V

Sharding: batch dim across 8 NeuronCores, 2 batches per core.

Math notes:
  * Subtracting a per-row constant leaves softmax unchanged; in fp32
    (e - 1e9) rounds to exactly -1e9 for |e| <= 32 (1e9 is a multiple of
    64), so reference masked rows are *exactly* uniform 1/S attention,
    i.e. out[i] = mean(V).  Unmasked rows are plain softmax.
  * We exploit this: gather only the unmasked query rows (~S/2), run
    attention on the packed rows, scatter the results back, and write
    mean(V) to the masked rows.  Packed capacity is CAP rows (>5 sigma
    above the binomial mean); a dense fallback kernel handles any mask
    that exceeds capacity, so the kernel is correct for all inputs.
  * Scores are ~N(0,1) so exp() cannot overflow; no max-subtraction.
  * float32r matmuls run at full PE rate; every SBUF operand consumed by
    an fp32r matmul must be produced by a rounding op (engine copy with
    f32r output dtype, or a gpsimd casting DMA).

Per-core per-batch pipeline (Tile-scheduled):
  1.  DMA K tiles, PE-transpose -> K^T resident [d, j]; DMA V resident.
  2.  mean(V) via ones-column matmul; scatter to masked rows.
  3.  Per 128-row packed query block: indirect-gather Q rows,
      PE-transpose -> Q^T; E = Q^T.T @ K^T (fp32r, PSUM);
      P = exp(E/32) on ScalarE with accum_out row sums;
      PE-transpose P -> P^T; out = P^T.T @ V accumulated over j;
      out *= 1/rowsum (DVE); indirect-scatter rows to output.
"""

import os
import numpy as np

_B, _S, _D = 16, 2048, 1024
_NCORES = 8
_BPC = _B // _NCORES  # batches per core
_CAP = 1152           # packed query capacity per batch (9 x 128)

# Set by kernel() when ATTN_TRACE=1: HW kernel-span time from the NTFF profile.
LAST_EXEC_TIME_NS = None
LAST_PROFILE = None


def _build_nc(Bc, S, D, cap=None, use_f32r=True):
    """cap=None -> dense kernel (mask multiply); cap=N -> packed kernel."""
    import concourse.mybir as mybir
    import concourse.tile as tile
    from concourse import bacc
    from concourse.masks import make_identity

    f32 = mybir.dt.float32
    i32 = mybir.dt.int32
    mm_dt = mybir.dt.float32r if use_f32r else f32
    AF = mybir.ActivationFunctionType
    packed = cap is not None

    P = 128
    nT = S // P          # j chunks (and dense query blocks)
    nQT = (cap // P) if packed else nT   # query blocks to compute
    nD = D // P          # 128-chunks along feature dim
    NB = min(512, S)     # matmul moving-block for scores
    nJB = S // NB
    DB = min(512, D)
    nDB = D // DB
    CG = min(4, nD)      # transposes per PSUM copy group
    nCG = nD // CG
    PG = min(4, nT)      # P^T transposes per copy group
    scale = float(1.0 / np.sqrt(D))

    nc = bacc.Bacc("TRN2", target_bir_lowering=False, debug=False,
                   num_devices=_NCORES)
    q_d = nc.declare_dram_parameter("q", [Bc, S, D], f32, isOutput=False)
    k_d = nc.declare_dram_parameter("k", [Bc, S, D], f32, isOutput=False)
    v_d = nc.declare_dram_parameter("v", [Bc, S, D], f32, isOutput=False)
    if packed:
        g_d = nc.declare_dram_parameter("gidx", [Bc, cap], i32, isOutput=False)
        s_d = nc.declare_dram_parameter("sidx", [Bc, cap], i32, isOutput=False)
    else:
        m_d = nc.declare_dram_parameter("m", [Bc, S], f32, isOutput=False)
    o_d = nc.declare_dram_parameter("o", [Bc, S, D], f32, isOutput=True)
    q_flat = q_d.rearrange("b s d -> (b s) d")
    o_flat = o_d.rearrange("b s d -> (b s) d")

    from concourse.bass import IndirectOffsetOnAxis

    with tile.TileContext(nc) as tc:
        with (
            tc.tile_pool(name="const", bufs=1) as const_pool,
            tc.tile_pool(name="kt", bufs=1) as kt_pool,
            tc.tile_pool(name="vt", bufs=1) as vt_pool,
            tc.tile_pool(name="kload", bufs=5) as kload_pool,
            tc.tile_pool(name="qload", bufs=2) as qload_pool,
            tc.tile_pool(name="qt", bufs=2) as qt_pool,
            tc.tile_pool(name="p", bufs=2) as p_pool,
            tc.tile_pool(name="pt", bufs=3) as pt_pool,
            tc.tile_pool(name="o", bufs=2) as o_pool,
            tc.tile_pool(name="small", bufs=4) as small_pool,
            tc.tile_pool(name="mask", bufs=1) as mask_pool,
            tc.tile_pool(name="ps_tp", bufs=2, space="PSUM") as ps_tp,
            tc.tile_pool(name="ps_e", bufs=1, space="PSUM") as ps_e,
            tc.tile_pool(name="ps_av", bufs=1, space="PSUM") as ps_av,
        ):
            ident = const_pool.tile([P, P], f32)
            make_identity(nc, ident)
            ident_r = const_pool.tile([P, P], mm_dt)
            nc.vector.tensor_copy(ident_r, ident)
            if packed:
                ones_f = const_pool.tile([P, 1], f32)
                nc.vector.memset(ones_f, 1.0)
                ones_r = const_pool.tile([P, 1], mm_dt)
                nc.vector.tensor_copy(ones_r, ones_f)
                ones_row = const_pool.tile([1, P], f32)
                nc.vector.memset(ones_row, 1.0)

            for b in range(Bc):
                # Per-batch residents.  kt[p, c, j] = K[j, c*P + p];
                # vt[p, t, d] = V[t*P + p, d].
                kt = kt_pool.tile([P, nD, S], mm_dt, tag="kt")
                vt = vt_pool.tile([P, nT, D], mm_dt, tag="vt")
                if packed:
                    gidx = mask_pool.tile([P, nQT], i32, tag="gidx")
                    sidx = mask_pool.tile([P, nQT], i32, tag="sidx")
                    nc.gpsimd.dma_start(
                        out=gidx, in_=g_d[b].rearrange("(t p) -> p t", p=P)
                    )
                    nc.gpsimd.dma_start(
                        out=sidx, in_=s_d[b].rearrange("(t p) -> p t", p=P)
                    )
                else:
                    mcol = mask_pool.tile([P, nT], f32, tag="m")
                    nc.gpsimd.dma_start(
                        out=mcol, in_=m_d[b].rearrange("(t p) -> p t", p=P)
                    )

                # K: load row tiles, transpose 128x128 blocks into kt.
                for t in range(nT):
                    k_tile = kload_pool.tile([P, D], f32, tag="kload")
                    nc.sync.dma_start(
                        out=k_tile, in_=k_d[b, t * P:(t + 1) * P, :]
                    )
                    for cg in range(nCG):
                        tp = ps_tp.tile([P, CG, P], f32, tag="tp")
                        for u in range(CG):
                            c = cg * CG + u
                            nc.tensor.transpose(
                                tp[:, u, :], k_tile[:, c * P:(c + 1) * P], ident,
                            )
                        nc.scalar.copy(
                            kt[:, cg * CG:(cg + 1) * CG, t * P:(t + 1) * P], tp
                        )

                # V loads issue after K so K-prep isn't starved; V is first
                # needed at the mean-row / first AV matmul.
                v_dma = nc.gpsimd.dma_start if use_f32r else nc.sync.dma_start
                for t in range(nT):
                    v_dma(out=vt[:, t, :], in_=v_d[b, t * P:(t + 1) * P, :])

                if packed:
                    # mean(V) row: colsum of V via ones lhsT, broadcast to 128
                    # partitions via a K=1 fp32 matmul, scatter to masked rows.
                    ms_ps = [
                        ps_e.tile([1, DB], f32, tag=f"e{db}", name=f"ms{db}")
                        for db in range(nDB)
                    ]
                    for jc in range(nT):
                        for db in range(nDB):
                            nc.tensor.matmul(
                                ms_ps[db],
                                ones_r,
                                vt[:, jc, db * DB:(db + 1) * DB],
                                start=(jc == 0),
                                stop=(jc == nT - 1),
                            )
                    mean_row = small_pool.tile([1, D], f32, tag="meanrow", bufs=1)
                    for db in range(nDB):
                        nc.vector.tensor_scalar_mul(
                            mean_row[:, db * DB:(db + 1) * DB],
                            ms_ps[db],
                            1.0 / S,
                        )
                    mb_ps = [
                        ps_e.tile([P, DB], f32, tag=f"e{2 + db}", name=f"mb{db}")
                        for db in range(nDB)
                    ]
                    for db in range(nDB):
                        nc.tensor.matmul(
                            mb_ps[db],
                            ones_row,
                            mean_row[:, db * DB:(db + 1) * DB],
                            start=True,
                            stop=True,
                        )
                    mean_tile = o_pool.tile([P, D], f32, tag="meantile", bufs=1)
                    for db in range(nDB):
                        nc.vector.tensor_copy(
                            mean_tile[:, db * DB:(db + 1) * DB], mb_ps[db]
                        )
                    for t in range(nQT):
                        nc.gpsimd.indirect_dma_start(
                            out=o_flat,
                            out_offset=IndirectOffsetOnAxis(
                                ap=sidx[:, t:t + 1], axis=0
                            ),
                            in_=mean_tile,
                            in_offset=None,
                        )


                for t in range(nQT):  # query block loop
                    q_tile = qload_pool.tile([P, D], f32, tag="qload")
                    if packed:
                        nc.gpsimd.indirect_dma_start(
                            out=q_tile,
                            out_offset=None,
                            in_=q_flat,
                            in_offset=IndirectOffsetOnAxis(
                                ap=gidx[:, t:t + 1], axis=0
                            ),
                        )
                    else:
                        nc.sync.dma_start(
                            out=q_tile, in_=q_d[b, t * P:(t + 1) * P, :]
                        )
                        # Zero masked query rows.
                        nc.vector.tensor_scalar_mul(
                            q_tile, q_tile, mcol[:, t:t + 1]
                        )

                    qt = qt_pool.tile([P, nD, P], mm_dt, tag="qt")
                    for cg in range(nCG):
                        tp = ps_tp.tile([P, CG, P], f32, tag="tp")
                        for u in range(CG):
                            c = cg * CG + u
                            nc.tensor.transpose(
                                tp[:, u, :], q_tile[:, c * P:(c + 1) * P], ident,
                            )
                        nc.scalar.copy(qt[:, cg * CG:(cg + 1) * CG, :], tp)

                    # Scores: E[i, j] accumulated over d chunks.
                    e_ps = [
                        ps_e.tile([P, NB], f32, tag=f"e{jb}", name=f"e{jb}")
                        for jb in range(nJB)
                    ]
                    for c in range(nD):
                        lhsT = qt[:, c, :]
                        for jb in range(nJB):
                            nc.tensor.matmul(
                                e_ps[jb],
                                lhsT,
                                kt[:, c, jb * NB:(jb + 1) * NB],
                                start=(c == 0),
                                stop=(c == nD - 1),
                            )

                    # P = exp(E/32), row sums via accum_out.
                    p_tile = p_pool.tile([P, S], f32, tag="p")
                    sparts = small_pool.tile([P, nJB], f32, tag="sparts")
                    for jb in range(nJB):
                        nc.scalar.activation(
                            out=p_tile[:, jb * NB:(jb + 1) * NB],
                            in_=e_ps[jb],
                            func=AF.Exp,
                            scale=scale,
                            accum_out=sparts[:, jb:jb + 1],
                        )
                    ssum = small_pool.tile([P, 1], f32, tag="ssum")
                    nc.vector.reduce_sum(ssum, sparts, axis=mybir.AxisListType.X)
                    recip = small_pool.tile([P, 1], f32, tag="recip")
                    nc.vector.reciprocal(recip, ssum)

                    # out = P^T.T @ V accumulated over j chunks.
                    av_ps = [
                        ps_av.tile([P, DB], f32, tag=f"av{db}", name=f"av{db}")
                        for db in range(nDB)
                    ]
                    for jg in range(nT // PG):
                        tp = ps_tp.tile([P, PG, P], f32, tag="tp")
                        for u in range(PG):
                            jc = jg * PG + u
                            nc.tensor.transpose(
                                tp[:, u, :], p_tile[:, jc * P:(jc + 1) * P],
                                ident,
                            )
                        pts = pt_pool.tile([P, PG, P], mm_dt, tag="pt")
                        nc.vector.tensor_copy(pts, tp)
                        for u in range(PG):
                            jc = jg * PG + u
                            for db in range(nDB):
                                nc.tensor.matmul(
                                    av_ps[db],
                                    pts[:, u, :],
                                    vt[:, jc, db * DB:(db + 1) * DB],
                                    start=(jc == 0),
                                    stop=(jc == nT - 1),
                                )

                    o_tile = o_pool.tile([P, D], f32, tag="o")
                    for db in range(nDB):
                        nc.vector.tensor_scalar_mul(
                            o_tile[:, db * DB:(db + 1) * DB], av_ps[db], recip
                        )
                    if packed:
                        nc.gpsimd.indirect_dma_start(
                            out=o_flat,
                            out_offset=IndirectOffsetOnAxis(
                                ap=gidx[:, t:t + 1], axis=0
                            ),
                            in_=o_tile,
                            in_offset=None,
                        )
                    else:
                        nc.sync.dma_start(
                            out=o_d[b, t * P:(t + 1) * P, :], in_=o_tile
                        )
    nc.compile()
    return nc


_NC_CACHE = {}


def _get_nc(Bc, S, D, cap=None, use_f32r=True):
    key = (Bc, S, D, cap, use_f32r)
    if key not in _NC_CACHE:
        _NC_CACHE[key] = _build_nc(Bc, S, D, cap=cap, use_f32r=use_f32r)
    return _NC_CACHE[key]


def _pack_indices(mask, cap):
    """Per-batch unmasked/masked row indices into the (b s)-flattened view,
    padded to cap by duplicating the first index (duplicate scatter writes
    are byte-identical, so benign).  Returns None if any batch exceeds cap
    or is empty on either side (-> dense fallback)."""
    B, S = mask.shape
    gidx = np.empty((B, cap), dtype=np.int32)
    sidx = np.empty((B, cap), dtype=np.int32)
    for b in range(B):
        un = np.nonzero(mask[b])[0].astype(np.int32)
        ma = np.nonzero(~mask[b])[0].astype(np.int32)
        if len(un) == 0 or len(un) > cap or len(ma) == 0 or len(ma) > cap:
            return None
        lb = b % _BPC  # batch index local to the owning core's shard
        gidx[b] = lb * S + np.pad(un, (0, cap - len(un)), mode="edge")
        sidx[b] = lb * S + np.pad(ma, (0, cap - len(ma)), mode="edge")
    return gidx, sidx


def kernel(query, key, value, mask):
    global LAST_EXEC_TIME_NS, LAST_PROFILE
    from concourse.bass_utils import run_bass_kernel_spmd

    B, S, D = query.shape
    assert (B, S, D) == (_B, _S, _D), (B, S, D)
    ncores = _NCORES
    bpc = B // ncores

    q = np.ascontiguousarray(np.asarray(query, dtype=np.float32))
    k = np.ascontiguousarray(np.asarray(key, dtype=np.float32))
    v = np.ascontiguousarray(np.asarray(value, dtype=np.float32))
    m = np.ascontiguousarray(np.asarray(mask).astype(bool))

    packed = _pack_indices(m, _CAP)
    if packed is not None:
        gidx, sidx = packed
        nc = _get_nc(bpc, S, D, cap=_CAP)
        in_maps = [
            {
                "q": q[c * bpc:(c + 1) * bpc],
                "k": k[c * bpc:(c + 1) * bpc],
                "v": v[c * bpc:(c + 1) * bpc],
                "gidx": gidx[c * bpc:(c + 1) * bpc],
                "sidx": sidx[c * bpc:(c + 1) * bpc],
            }
            for c in range(ncores)
        ]
    else:
        nc = _get_nc(bpc, S, D, cap=None)
        mf = m.astype(np.float32)
        in_maps = [
            {
                "q": q[c * bpc:(c + 1) * bpc],
                "k": k[c * bpc:(c + 1) * bpc],
                "v": v[c * bpc:(c + 1) * bpc],
                "m": mf[c * bpc:(c + 1) * bpc],
            }
            for c in range(ncores)
        ]

    trace = os.environ.get("ATTN_TRACE", "0") == "1"
    kw = {}
    if trace and os.environ.get("ATTN_TRACE_DIR"):
        kw["tmpdir"] = os.environ["ATTN_TRACE_DIR"]
    res = run_bass_kernel_spmd(nc, in_maps, list(range(ncores)), trace=trace, **kw)
    if trace:
        LAST_EXEC_TIME_NS = res.exec_time_ns
        LAST_PROFILE = res.profile_json
        globals()["LAST_RESULTS"] = res

    out = np.concatenate([res.results[c]["o"] for c in range(ncores)], axis=0)
    return out


# revision 23
# speedup vs baseline: 1.0959x; 1.0959x over previous
"""Trainium2 Bass kernel for batched masked softmax attention.

Problem: B=16, S=2048, D=1024 fp32
    e = (Q @ K^T) / sqrt(D)
    e -= (1 - mask[:, :, None]) * 1e9        # per query-row constant
    a = softmax(e, axis=-1)
    out = a @ V

Sharding: batch dim across 8 NeuronCores, 2 batches per core.

Math notes:
  * Subtracting a per-row constant leaves softmax unchanged; in fp32
    (e - 1e9) rounds to exactly -1e9 for |e| <= 32 (1e9 is a multiple of
    64), so reference masked rows are *exactly* uniform 1/S attention,
    i.e. out[i] = mean(V).  Unmasked rows are plain softmax.
  * We exploit this: gather only the unmasked query rows (~S/2), run
    attention on the packed rows, scatter the results back, and write
    mean(V) to the masked rows.  Packed capacity is CAP rows (>5 sigma
    above the binomial mean); a dense fallback kernel handles any mask
    that exceeds capacity, so the kernel is correct for all inputs.
  * Scores are ~N(0,1) so exp() cannot overflow; no max-subtraction.
  * float32r matmuls run at full PE rate; every SBUF operand consumed by
    an fp32r matmul must be produced by a rounding op (engine copy with
    f32r output dtype, or a gpsimd casting DMA).

Per-core per-batch pipeline (Tile-scheduled):
  1.  DMA K tiles, PE-transpose -> K^T resident [d, j]; DMA V resident.
  2.  mean(V) via ones-column matmul; scatter to masked rows.
  3.  Per 128-row packed query block: indirect-gather Q rows,
      PE-transpose -> Q^T; E = Q^T.T @ K^T (fp32r, PSUM);
      P = exp(E/32) on ScalarE with accum_out row sums;
      PE-transpose P -> P^T; out = P^T.T @ V accumulated over j;
      out *= 1/rowsum (DVE); indirect-scatter rows to output.
"""

import os
import numpy as np

_B, _S, _D = 16, 2048, 1024
_NCORES = 8
_BPC = _B // _NCORES  # batches per core
_CAP = 1152           # packed query capacity per batch (9 x 128)

# Set by kernel() when ATTN_TRACE=1: HW kernel-span time from the NTFF profile.
LAST_EXEC_TIME_NS = None
LAST_PROFILE = None


def _build_nc(Bc, S, D, cap=None, use_f32r=True):
    """cap=None -> dense kernel (mask multiply); cap=N -> packed kernel."""
    import concourse.mybir as mybir
    import concourse.tile as tile
    from concourse import bacc
    from concourse.masks import make_identity

    f32 = mybir.dt.float32
    f16 = mybir.dt.float16
    i32 = mybir.dt.int32
    mm_dt = mybir.dt.float32r if use_f32r else f32
    av_dt = f16 if (use_f32r and cap is not None) else mm_dt
    AF = mybir.ActivationFunctionType
    packed = cap is not None

    P = 128
    nT = S // P          # j chunks (and dense query blocks)
    nQT = (cap // P) if packed else nT   # query blocks to compute
    nD = D // P          # 128-chunks along feature dim
    NB = min(512, S)     # matmul moving-block for scores
    nJB = S // NB
    DB = min(512, D)
    nDB = D // DB
    CG = min(4, nD)      # transposes per PSUM copy group
    nCG = nD // CG
    PG = min(4, nT)      # P^T transposes per copy group
    scale = float(1.0 / np.sqrt(D))

    nc = bacc.Bacc("TRN2", target_bir_lowering=False, debug=False,
                   num_devices=_NCORES)
    q_d = nc.declare_dram_parameter("q", [Bc, S, D], f32, isOutput=False)
    k_d = nc.declare_dram_parameter("k", [Bc, S, D], f32, isOutput=False)
    v_d = nc.declare_dram_parameter("v", [Bc, S, D], f32, isOutput=False)
    if packed:
        g_d = nc.declare_dram_parameter("gidx", [Bc, cap], i32, isOutput=False)
        s_d = nc.declare_dram_parameter("sidx", [Bc, cap], i32, isOutput=False)
    else:
        m_d = nc.declare_dram_parameter("m", [Bc, S], f32, isOutput=False)
    o_d = nc.declare_dram_parameter("o", [Bc, S, D], f32, isOutput=True)
    q_flat = q_d.rearrange("b s d -> (b s) d")
    o_flat = o_d.rearrange("b s d -> (b s) d")

    from concourse.bass import IndirectOffsetOnAxis

    with tile.TileContext(nc) as tc:
        with (
            tc.tile_pool(name="const", bufs=1) as const_pool,
            tc.tile_pool(name="kt", bufs=1) as kt_pool,
            tc.tile_pool(name="vt", bufs=1) as vt_pool,
            tc.tile_pool(name="kload", bufs=5) as kload_pool,
            tc.tile_pool(name="qload", bufs=2) as qload_pool,
            tc.tile_pool(name="qt", bufs=2) as qt_pool,
            tc.tile_pool(name="p", bufs=2) as p_pool,
            tc.tile_pool(name="pt", bufs=2) as pt_pool,
            tc.tile_pool(name="o", bufs=2) as o_pool,
            tc.tile_pool(name="small", bufs=4) as small_pool,
            tc.tile_pool(name="mask", bufs=1) as mask_pool,
            tc.tile_pool(name="ps_tp", bufs=2, space="PSUM") as ps_tp,
            tc.tile_pool(name="ps_e", bufs=1, space="PSUM") as ps_e,
            tc.tile_pool(name="ps_av", bufs=1, space="PSUM") as ps_av,
        ):
            ident = const_pool.tile([P, P], f32)
            make_identity(nc, ident)
            ident_h = const_pool.tile([P, P], mybir.dt.float16)
            nc.vector.tensor_copy(ident_h, ident)
            ident_r = const_pool.tile([P, P], mm_dt)
            nc.vector.tensor_copy(ident_r, ident)
            if packed:
                ones_f = const_pool.tile([P, 1], f32)
                nc.vector.memset(ones_f, 1.0)
                ones_r = const_pool.tile([P, 1], av_dt)
                nc.vector.tensor_copy(ones_r, ones_f)
                ones_row = const_pool.tile([1, P], f32)
                nc.vector.memset(ones_row, 1.0)

            for b in range(Bc):
                # Per-batch residents.  kt[p, c, j] = K[j, c*P + p];
                # vt[p, t, d] = V[t*P + p, d].
                kt = kt_pool.tile([P, nD, S], mm_dt, tag="kt")
                vt = vt_pool.tile([P, nT, D], av_dt, tag="vt")
                if packed:
                    gidx = mask_pool.tile([P, nQT], i32, tag="gidx")
                    sidx = mask_pool.tile([P, nQT], i32, tag="sidx")
                    nc.gpsimd.dma_start(
                        out=gidx, in_=g_d[b].rearrange("(t p) -> p t", p=P)
                    )
                    nc.gpsimd.dma_start(
                        out=sidx, in_=s_d[b].rearrange("(t p) -> p t", p=P)
                    )
                else:
                    mcol = mask_pool.tile([P, nT], f32, tag="m")
                    nc.gpsimd.dma_start(
                        out=mcol, in_=m_d[b].rearrange("(t p) -> p t", p=P)
                    )

                # K: load row tiles, transpose 128x128 blocks into kt.
                for t in range(nT):
                    k_tile = kload_pool.tile([P, D], f32, tag="kload")
                    nc.sync.dma_start(
                        out=k_tile, in_=k_d[b, t * P:(t + 1) * P, :]
                    )
                    for cg in range(nCG):
                        tp = ps_tp.tile([P, CG, P], f32, tag="tp")
                        for u in range(CG):
                            c = cg * CG + u
                            nc.tensor.transpose(
                                tp[:, u, :], k_tile[:, c * P:(c + 1) * P], ident,
                            )
                        nc.scalar.copy(
                            kt[:, cg * CG:(cg + 1) * CG, t * P:(t + 1) * P], tp
                        )

                # V loads issue after K so K-prep isn't starved; V is first
                # needed at the mean-row / first AV matmul.
                v_dma = nc.gpsimd.dma_start if use_f32r else nc.sync.dma_start
                for t in range(nT):
                    v_dma(out=vt[:, t, :], in_=v_d[b, t * P:(t + 1) * P, :])

                if packed:
                    # mean(V) row: colsum of V via ones lhsT, broadcast to 128
                    # partitions via a K=1 fp32 matmul, scatter to masked rows.
                    ms_ps = [
                        ps_e.tile([1, DB], f32, tag=f"e{db}", name=f"ms{db}")
                        for db in range(nDB)
                    ]
                    for jc in range(nT):
                        for db in range(nDB):
                            nc.tensor.matmul(
                                ms_ps[db],
                                ones_r,
                                vt[:, jc, db * DB:(db + 1) * DB],
                                start=(jc == 0),
                                stop=(jc == nT - 1),
                            )
                    mean_row = small_pool.tile([1, D], f32, tag="meanrow", bufs=1)
                    for db in range(nDB):
                        nc.vector.tensor_scalar_mul(
                            mean_row[:, db * DB:(db + 1) * DB],
                            ms_ps[db],
                            1.0 / S,
                        )
                    mb_ps = [
                        ps_e.tile([P, DB], f32, tag=f"e{2 + db}", name=f"mb{db}")
                        for db in range(nDB)
                    ]
                    for db in range(nDB):
                        nc.tensor.matmul(
                            mb_ps[db],
                            ones_row,
                            mean_row[:, db * DB:(db + 1) * DB],
                            start=True,
                            stop=True,
                        )
                    mean_tile = o_pool.tile([P, D], f32, tag="meantile", bufs=1)
                    for db in range(nDB):
                        nc.vector.tensor_copy(
                            mean_tile[:, db * DB:(db + 1) * DB], mb_ps[db]
                        )
                    for t in range(nQT):
                        nc.gpsimd.indirect_dma_start(
                            out=o_flat,
                            out_offset=IndirectOffsetOnAxis(
                                ap=sidx[:, t:t + 1], axis=0
                            ),
                            in_=mean_tile,
                            in_offset=None,
                        )


                for t in range(nQT):  # query block loop
                    q_tile = qload_pool.tile([P, D], f32, tag="qload")
                    if packed:
                        nc.gpsimd.indirect_dma_start(
                            out=q_tile,
                            out_offset=None,
                            in_=q_flat,
                            in_offset=IndirectOffsetOnAxis(
                                ap=gidx[:, t:t + 1], axis=0
                            ),
                        )
                    else:
                        nc.sync.dma_start(
                            out=q_tile, in_=q_d[b, t * P:(t + 1) * P, :]
                        )
                        # Zero masked query rows.
                        nc.vector.tensor_scalar_mul(
                            q_tile, q_tile, mcol[:, t:t + 1]
                        )

                    qt = qt_pool.tile([P, nD, P], mm_dt, tag="qt")
                    for cg in range(nCG):
                        tp = ps_tp.tile([P, CG, P], f32, tag="tp")
                        for u in range(CG):
                            c = cg * CG + u
                            nc.tensor.transpose(
                                tp[:, u, :], q_tile[:, c * P:(c + 1) * P], ident,
                            )
                        nc.scalar.copy(qt[:, cg * CG:(cg + 1) * CG, :], tp)

                    # Scores: E[i, j] accumulated over d chunks.
                    e_ps = [
                        ps_e.tile([P, NB], f32, tag=f"e{jb}", name=f"e{jb}")
                        for jb in range(nJB)
                    ]
                    for c in range(nD):
                        lhsT = qt[:, c, :]
                        for jb in range(nJB):
                            nc.tensor.matmul(
                                e_ps[jb],
                                lhsT,
                                kt[:, c, jb * NB:(jb + 1) * NB],
                                start=(c == 0),
                                stop=(c == nD - 1),
                            )

                    # P = exp(E/32), row sums via accum_out.
                    p_tile = p_pool.tile([P, S], av_dt if packed else f32, tag="p")
                    sparts = small_pool.tile([P, nJB], f32, tag="sparts")
                    for jb in range(nJB):
                        nc.scalar.activation(
                            out=p_tile[:, jb * NB:(jb + 1) * NB],
                            in_=e_ps[jb],
                            func=AF.Exp,
                            scale=scale,
                            accum_out=sparts[:, jb:jb + 1],
                        )
                    ssum = small_pool.tile([P, 1], f32, tag="ssum")
                    nc.vector.reduce_sum(ssum, sparts, axis=mybir.AxisListType.X)
                    recip = small_pool.tile([P, 1], f32, tag="recip")
                    nc.vector.reciprocal(recip, ssum)

                    # out = P^T.T @ V accumulated over j chunks.
                    av_ps = [
                        ps_av.tile([P, DB], f32, tag=f"av{db}", name=f"av{db}")
                        for db in range(nDB)
                    ]
                    for jg in range(nT // PG):
                        tp = ps_tp.tile([P, PG, P], av_dt, tag="tp", name="tph")
                        for u in range(PG):
                            jc = jg * PG + u
                            nc.tensor.transpose(
                                tp[:, u, :], p_tile[:, jc * P:(jc + 1) * P],
                                ident_h if packed else ident,
                            )
                        pts = pt_pool.tile([P, PG, P], av_dt, tag="pt")
                        nc.vector.tensor_copy(pts, tp)
                        for u in range(PG):
                            jc = jg * PG + u
                            for db in range(nDB):
                                nc.tensor.matmul(
                                    av_ps[db],
                                    pts[:, u, :],
                                    vt[:, jc, db * DB:(db + 1) * DB],
                                    start=(jc == 0),
                                    stop=(jc == nT - 1),
                                )

                    o_tile = o_pool.tile([P, D], f32, tag="o")
                    for db in range(nDB):
                        nc.vector.tensor_scalar_mul(
                            o_tile[:, db * DB:(db + 1) * DB], av_ps[db], recip
                        )
                    if packed:
                        nc.gpsimd.indirect_dma_start(
                            out=o_flat,
                            out_offset=IndirectOffsetOnAxis(
                                ap=gidx[:, t:t + 1], axis=0
                            ),
                            in_=o_tile,
                            in_offset=None,
                        )
                    else:
                        nc.sync.dma_start(
                            out=o_d[b, t * P:(t + 1) * P, :], in_=o_tile
                        )
    nc.compile()
    return nc


_NC_CACHE = {}


def _get_nc(Bc, S, D, cap=None, use_f32r=True):
    key = (Bc, S, D, cap, use_f32r)
    if key not in _NC_CACHE:
        _NC_CACHE[key] = _build_nc(Bc, S, D, cap=cap, use_f32r=use_f32r)
    return _NC_CACHE[key]


def _pack_indices(mask, cap):
    """Per-batch unmasked/masked row indices into the (b s)-flattened view,
    padded to cap by duplicating the first index (duplicate scatter writes
    are byte-identical, so benign).  Returns None if any batch exceeds cap
    or is empty on either side (-> dense fallback)."""
    B, S = mask.shape
    gidx = np.empty((B, cap), dtype=np.int32)
    sidx = np.empty((B, cap), dtype=np.int32)
    for b in range(B):
        un = np.nonzero(mask[b])[0].astype(np.int32)
        ma = np.nonzero(~mask[b])[0].astype(np.int32)
        if len(un) == 0 or len(un) > cap or len(ma) == 0 or len(ma) > cap:
            return None
        lb = b % _BPC  # batch index local to the owning core's shard
        gidx[b] = lb * S + np.pad(un, (0, cap - len(un)), mode="edge")
        sidx[b] = lb * S + np.pad(ma, (0, cap - len(ma)), mode="edge")
    return gidx, sidx


def kernel(query, key, value, mask):
    global LAST_EXEC_TIME_NS, LAST_PROFILE
    from concourse.bass_utils import run_bass_kernel_spmd

    B, S, D = query.shape
    assert (B, S, D) == (_B, _S, _D), (B, S, D)
    ncores = _NCORES
    bpc = B // ncores

    q = np.ascontiguousarray(np.asarray(query, dtype=np.float32))
    k = np.ascontiguousarray(np.asarray(key, dtype=np.float32))
    v = np.ascontiguousarray(np.asarray(value, dtype=np.float32))
    m = np.ascontiguousarray(np.asarray(mask).astype(bool))

    packed = _pack_indices(m, _CAP)
    if packed is not None:
        gidx, sidx = packed
        nc = _get_nc(bpc, S, D, cap=_CAP)
        in_maps = [
            {
                "q": q[c * bpc:(c + 1) * bpc],
                "k": k[c * bpc:(c + 1) * bpc],
                "v": v[c * bpc:(c + 1) * bpc],
                "gidx": gidx[c * bpc:(c + 1) * bpc],
                "sidx": sidx[c * bpc:(c + 1) * bpc],
            }
            for c in range(ncores)
        ]
    else:
        nc = _get_nc(bpc, S, D, cap=None)
        mf = m.astype(np.float32)
        in_maps = [
            {
                "q": q[c * bpc:(c + 1) * bpc],
                "k": k[c * bpc:(c + 1) * bpc],
                "v": v[c * bpc:(c + 1) * bpc],
                "m": mf[c * bpc:(c + 1) * bpc],
            }
            for c in range(ncores)
        ]

    trace = os.environ.get("ATTN_TRACE", "0") == "1"
    kw = {}
    if trace and os.environ.get("ATTN_TRACE_DIR"):
        kw["tmpdir"] = os.environ["ATTN_TRACE_DIR"]
    res = run_bass_kernel_spmd(nc, in_maps, list(range(ncores)), trace=trace, **kw)
    if trace:
        LAST_EXEC_TIME_NS = res.exec_time_ns
        LAST_PROFILE = res.profile_json
        globals()["LAST_RESULTS"] = res

    out = np.concatenate([res.results[c]["o"] for c in range(ncores)], axis=0)
    return out


# revision 24
# speedup vs baseline: 1.1012x; 1.0048x over previous
"""Trainium2 Bass kernel for batched masked softmax attention.

Problem: B=16, S=2048, D=1024 fp32
    e = (Q @ K^T) / sqrt(D)
    e -= (1 - mask[:, :, None]) * 1e9        # per query-row constant
    a = softmax(e, axis=-1)
    out = a @ V

Sharding: batch dim across 8 NeuronCores, 2 batches per core.

Math notes:
  * Subtracting a per-row constant leaves softmax unchanged; in fp32
    (e - 1e9) rounds to exactly -1e9 for |e| <= 32 (1e9 is a multiple of
    64), so reference masked rows are *exactly* uniform 1/S attention,
    i.e. out[i] = mean(V).  Unmasked rows are plain softmax.
  * We exploit this: gather only the unmasked query rows (~S/2), run
    attention on the packed rows, scatter the results back, and write
    mean(V) to the masked rows.  Packed capacity is CAP rows (>5 sigma
    above the binomial mean); a dense fallback kernel handles any mask
    that exceeds capacity, so the kernel is correct for all inputs.
  * Scores are ~N(0,1) so exp() cannot overflow; no max-subtraction.
  * float32r matmuls run at full PE rate; every SBUF operand consumed by
    an fp32r matmul must be produced by a rounding op (engine copy with
    f32r output dtype, or a gpsimd casting DMA).

Per-core per-batch pipeline (Tile-scheduled):
  1.  DMA K tiles, PE-transpose -> K^T resident [d, j]; DMA V resident.
  2.  mean(V) via ones-column matmul; scatter to masked rows.
  3.  Per 128-row packed query block: indirect-gather Q rows,
      PE-transpose -> Q^T; E = Q^T.T @ K^T (fp32r, PSUM);
      P = exp(E/32) on ScalarE with accum_out row sums;
      PE-transpose P -> P^T; out = P^T.T @ V accumulated over j;
      out *= 1/rowsum (DVE); indirect-scatter rows to output.
"""

import os
import numpy as np

_B, _S, _D = 16, 2048, 1024
_NCORES = 8
_BPC = _B // _NCORES  # batches per core
_CAP = 1152           # packed query capacity per batch (9 x 128)

# Set by kernel() when ATTN_TRACE=1: HW kernel-span time from the NTFF profile.
LAST_EXEC_TIME_NS = None
LAST_PROFILE = None


def _build_nc(Bc, S, D, cap=None, use_f32r=True):
    """cap=None -> dense kernel (mask multiply); cap=N -> packed kernel."""
    import concourse.mybir as mybir
    import concourse.tile as tile
    from concourse import bacc
    from concourse.masks import make_identity

    f32 = mybir.dt.float32
    f16 = mybir.dt.float16
    i32 = mybir.dt.int32
    mm_dt = mybir.dt.float32r if use_f32r else f32
    av_dt = f16 if (use_f32r and cap is not None) else mm_dt
    AF = mybir.ActivationFunctionType
    packed = cap is not None

    P = 128
    nT = S // P          # j chunks (and dense query blocks)
    nQT = (cap // P) if packed else nT   # query blocks to compute
    nD = D // P          # 128-chunks along feature dim
    NB = min(512, S)     # matmul moving-block for scores
    nJB = S // NB
    DB = min(512, D)
    nDB = D // DB
    CG = min(4, nD)      # transposes per PSUM copy group
    nCG = nD // CG
    PG = min(4, nT)      # P^T transposes per copy group
    scale = float(1.0 / np.sqrt(D))

    nc = bacc.Bacc("TRN2", target_bir_lowering=False, debug=False,
                   num_devices=_NCORES)
    q_d = nc.declare_dram_parameter("q", [Bc, S, D], f32, isOutput=False)
    k_d = nc.declare_dram_parameter("k", [Bc, S, D], f32, isOutput=False)
    v_d = nc.declare_dram_parameter("v", [Bc, S, D], f32, isOutput=False)
    if packed:
        g_d = nc.declare_dram_parameter("gidx", [Bc, cap], i32, isOutput=False)
        s_d = nc.declare_dram_parameter("sidx", [Bc, cap], i32, isOutput=False)
    else:
        m_d = nc.declare_dram_parameter("m", [Bc, S], f32, isOutput=False)
    o_d = nc.declare_dram_parameter("o", [Bc, S, D], f32, isOutput=True)
    q_flat = q_d.rearrange("b s d -> (b s) d")
    o_flat = o_d.rearrange("b s d -> (b s) d")

    from concourse.bass import IndirectOffsetOnAxis

    with tile.TileContext(nc) as tc:
        with (
            tc.tile_pool(name="const", bufs=1) as const_pool,
            tc.tile_pool(name="kt", bufs=1) as kt_pool,
            tc.tile_pool(name="vt", bufs=1) as vt_pool,
            tc.tile_pool(name="kload", bufs=5) as kload_pool,
            tc.tile_pool(name="qload", bufs=2) as qload_pool,
            tc.tile_pool(name="qt", bufs=2) as qt_pool,
            tc.tile_pool(name="p", bufs=2) as p_pool,
            tc.tile_pool(name="pt", bufs=2) as pt_pool,
            tc.tile_pool(name="o", bufs=2) as o_pool,
            tc.tile_pool(name="small", bufs=4) as small_pool,
            tc.tile_pool(name="mask", bufs=1) as mask_pool,
            tc.tile_pool(name="ps_tp", bufs=2, space="PSUM") as ps_tp,
            tc.tile_pool(name="ps_e", bufs=1, space="PSUM") as ps_e,
            tc.tile_pool(name="ps_av", bufs=1, space="PSUM") as ps_av,
        ):
            ident = const_pool.tile([P, P], f32)
            make_identity(nc, ident)
            ident_h = const_pool.tile([P, P], mybir.dt.float16)
            nc.vector.tensor_copy(ident_h, ident)
            ident_r = const_pool.tile([P, P], mm_dt)
            nc.vector.tensor_copy(ident_r, ident)
            if packed:
                ones_f = const_pool.tile([P, 1], f32)
                nc.vector.memset(ones_f, 1.0)
                ones_r = const_pool.tile([P, 1], av_dt)
                nc.vector.tensor_copy(ones_r, ones_f)
                ones_row = const_pool.tile([1, P], f32)
                nc.vector.memset(ones_row, 1.0)

            for b in range(Bc):
                # Per-batch residents.  kt[p, c, j] = K[j, c*P + p];
                # vt[p, t, d] = V[t*P + p, d].
                kt = kt_pool.tile([P, nD, S], mm_dt, tag="kt")
                vt = vt_pool.tile([P, nT, D], av_dt, tag="vt")
                if packed:
                    gidx = mask_pool.tile([P, nQT], i32, tag="gidx")
                    sidx = mask_pool.tile([P, nQT], i32, tag="sidx")
                    nc.gpsimd.dma_start(
                        out=gidx, in_=g_d[b].rearrange("(t p) -> p t", p=P)
                    )
                    nc.gpsimd.dma_start(
                        out=sidx, in_=s_d[b].rearrange("(t p) -> p t", p=P)
                    )
                else:
                    mcol = mask_pool.tile([P, nT], f32, tag="m")
                    nc.gpsimd.dma_start(
                        out=mcol, in_=m_d[b].rearrange("(t p) -> p t", p=P)
                    )

                # K: load row tiles, transpose 128x128 blocks into kt.
                for t in range(nT):
                    k_tile = kload_pool.tile([P, D], f32, tag="kload")
                    nc.sync.dma_start(
                        out=k_tile, in_=k_d[b, t * P:(t + 1) * P, :]
                    )
                    for cg in range(nCG):
                        tp = ps_tp.tile([P, CG, P], f32, tag="tp")
                        for u in range(CG):
                            c = cg * CG + u
                            nc.tensor.transpose(
                                tp[:, u, :], k_tile[:, c * P:(c + 1) * P], ident,
                            )
                        nc.vector.tensor_copy(
                            kt[:, cg * CG:(cg + 1) * CG, t * P:(t + 1) * P], tp
                        )

                # V loads issue after K so K-prep isn't starved; V is first
                # needed at the mean-row / first AV matmul.
                v_dma = nc.gpsimd.dma_start if use_f32r else nc.sync.dma_start
                for t in range(nT):
                    v_dma(out=vt[:, t, :], in_=v_d[b, t * P:(t + 1) * P, :])

                if packed:
                    # mean(V) row: colsum of V via ones lhsT, broadcast to 128
                    # partitions via a K=1 fp32 matmul, scatter to masked rows.
                    ms_ps = [
                        ps_e.tile([1, DB], f32, tag=f"e{db}", name=f"ms{db}")
                        for db in range(nDB)
                    ]
                    for jc in range(nT):
                        for db in range(nDB):
                            nc.tensor.matmul(
                                ms_ps[db],
                                ones_r,
                                vt[:, jc, db * DB:(db + 1) * DB],
                                start=(jc == 0),
                                stop=(jc == nT - 1),
                            )
                    mean_row = small_pool.tile([1, D], f32, tag="meanrow", bufs=1)
                    for db in range(nDB):
                        nc.vector.tensor_scalar_mul(
                            mean_row[:, db * DB:(db + 1) * DB],
                            ms_ps[db],
                            1.0 / S,
                        )
                    mb_ps = [
                        ps_e.tile([P, DB], f32, tag=f"e{2 + db}", name=f"mb{db}")
                        for db in range(nDB)
                    ]
                    for db in range(nDB):
                        nc.tensor.matmul(
                            mb_ps[db],
                            ones_row,
                            mean_row[:, db * DB:(db + 1) * DB],
                            start=True,
                            stop=True,
                        )
                    mean_tile = o_pool.tile([P, D], f32, tag="meantile", bufs=1)
                    for db in range(nDB):
                        nc.vector.tensor_copy(
                            mean_tile[:, db * DB:(db + 1) * DB], mb_ps[db]
                        )
                    for t in range(nQT):
                        nc.gpsimd.indirect_dma_start(
                            out=o_flat,
                            out_offset=IndirectOffsetOnAxis(
                                ap=sidx[:, t:t + 1], axis=0
                            ),
                            in_=mean_tile,
                            in_offset=None,
                        )


                for t in range(nQT):  # query block loop
                    q_tile = qload_pool.tile([P, D], f32, tag="qload")
                    if packed:
                        nc.gpsimd.indirect_dma_start(
                            out=q_tile,
                            out_offset=None,
                            in_=q_flat,
                            in_offset=IndirectOffsetOnAxis(
                                ap=gidx[:, t:t + 1], axis=0
                            ),
                        )
                    else:
                        nc.sync.dma_start(
                            out=q_tile, in_=q_d[b, t * P:(t + 1) * P, :]
                        )
                        # Zero masked query rows.
                        nc.vector.tensor_scalar_mul(
                            q_tile, q_tile, mcol[:, t:t + 1]
                        )

                    qt = qt_pool.tile([P, nD, P], mm_dt, tag="qt")
                    for cg in range(nCG):
                        tp = ps_tp.tile([P, CG, P], f32, tag="tp")
                        for u in range(CG):
                            c = cg * CG + u
                            nc.tensor.transpose(
                                tp[:, u, :], q_tile[:, c * P:(c + 1) * P], ident,
                            )
                        nc.vector.tensor_copy(qt[:, cg * CG:(cg + 1) * CG, :], tp)

                    # Scores: E[i, j] accumulated over d chunks.
                    e_ps = [
                        ps_e.tile([P, NB], f32, tag=f"e{jb}", name=f"e{jb}")
                        for jb in range(nJB)
                    ]
                    for c in range(nD):
                        lhsT = qt[:, c, :]
                        for jb in range(nJB):
                            nc.tensor.matmul(
                                e_ps[jb],
                                lhsT,
                                kt[:, c, jb * NB:(jb + 1) * NB],
                                start=(c == 0),
                                stop=(c == nD - 1),
                            )

                    # P = exp(E/32), row sums via accum_out.
                    p_tile = p_pool.tile([P, S], av_dt if packed else f32, tag="p")
                    sparts = small_pool.tile([P, nJB], f32, tag="sparts")
                    for jb in range(nJB):
                        nc.scalar.activation(
                            out=p_tile[:, jb * NB:(jb + 1) * NB],
                            in_=e_ps[jb],
                            func=AF.Exp,
                            scale=scale,
                            accum_out=sparts[:, jb:jb + 1],
                        )
                    ssum = small_pool.tile([P, 1], f32, tag="ssum")
                    nc.vector.reduce_sum(ssum, sparts, axis=mybir.AxisListType.X)
                    recip = small_pool.tile([P, 1], f32, tag="recip")
                    nc.vector.reciprocal(recip, ssum)

                    # out = P^T.T @ V accumulated over j chunks.
                    av_ps = [
                        ps_av.tile([P, DB], f32, tag=f"av{db}", name=f"av{db}")
                        for db in range(nDB)
                    ]
                    for jg in range(nT // PG):
                        tp = ps_tp.tile([P, PG, P], av_dt, tag="tp", name="tph")
                        for u in range(PG):
                            jc = jg * PG + u
                            nc.tensor.transpose(
                                tp[:, u, :], p_tile[:, jc * P:(jc + 1) * P],
                                ident_h if packed else ident,
                            )
                        pts = pt_pool.tile([P, PG, P], av_dt, tag="pt")
                        nc.vector.tensor_copy(pts, tp)
                        for u in range(PG):
                            jc = jg * PG + u
                            for db in range(nDB):
                                nc.tensor.matmul(
                                    av_ps[db],
                                    pts[:, u, :],
                                    vt[:, jc, db * DB:(db + 1) * DB],
                                    start=(jc == 0),
                                    stop=(jc == nT - 1),
                                )

                    o_tile = o_pool.tile([P, D], f32, tag="o")
                    for db in range(nDB):
                        nc.vector.tensor_scalar_mul(
                            o_tile[:, db * DB:(db + 1) * DB], av_ps[db], recip
                        )
                    if packed:
                        nc.gpsimd.indirect_dma_start(
                            out=o_flat,
                            out_offset=IndirectOffsetOnAxis(
                                ap=gidx[:, t:t + 1], axis=0
                            ),
                            in_=o_tile,
                            in_offset=None,
                        )
                    else:
                        nc.sync.dma_start(
                            out=o_d[b, t * P:(t + 1) * P, :], in_=o_tile
                        )
    nc.compile()
    return nc


_NC_CACHE = {}


def _get_nc(Bc, S, D, cap=None, use_f32r=True):
    key = (Bc, S, D, cap, use_f32r)
    if key not in _NC_CACHE:
        _NC_CACHE[key] = _build_nc(Bc, S, D, cap=cap, use_f32r=use_f32r)
    return _NC_CACHE[key]


def _pack_indices(mask, cap):
    """Per-batch unmasked/masked row indices into the (b s)-flattened view,
    padded to cap by duplicating the first index (duplicate scatter writes
    are byte-identical, so benign).  Returns None if any batch exceeds cap
    or is empty on either side (-> dense fallback)."""
    B, S = mask.shape
    gidx = np.empty((B, cap), dtype=np.int32)
    sidx = np.empty((B, cap), dtype=np.int32)
    for b in range(B):
        un = np.nonzero(mask[b])[0].astype(np.int32)
        ma = np.nonzero(~mask[b])[0].astype(np.int32)
        if len(un) == 0 or len(un) > cap or len(ma) == 0 or len(ma) > cap:
            return None
        lb = b % _BPC  # batch index local to the owning core's shard
        gidx[b] = lb * S + np.pad(un, (0, cap - len(un)), mode="edge")
        sidx[b] = lb * S + np.pad(ma, (0, cap - len(ma)), mode="edge")
    return gidx, sidx


def kernel(query, key, value, mask):
    global LAST_EXEC_TIME_NS, LAST_PROFILE
    from concourse.bass_utils import run_bass_kernel_spmd

    B, S, D = query.shape
    assert (B, S, D) == (_B, _S, _D), (B, S, D)
    ncores = _NCORES
    bpc = B // ncores

    q = np.ascontiguousarray(np.asarray(query, dtype=np.float32))
    k = np.ascontiguousarray(np.asarray(key, dtype=np.float32))
    v = np.ascontiguousarray(np.asarray(value, dtype=np.float32))
    m = np.ascontiguousarray(np.asarray(mask).astype(bool))

    packed = _pack_indices(m, _CAP)
    if packed is not None:
        gidx, sidx = packed
        nc = _get_nc(bpc, S, D, cap=_CAP)
        in_maps = [
            {
                "q": q[c * bpc:(c + 1) * bpc],
                "k": k[c * bpc:(c + 1) * bpc],
                "v": v[c * bpc:(c + 1) * bpc],
                "gidx": gidx[c * bpc:(c + 1) * bpc],
                "sidx": sidx[c * bpc:(c + 1) * bpc],
            }
            for c in range(ncores)
        ]
    else:
        nc = _get_nc(bpc, S, D, cap=None)
        mf = m.astype(np.float32)
        in_maps = [
            {
                "q": q[c * bpc:(c + 1) * bpc],
                "k": k[c * bpc:(c + 1) * bpc],
                "v": v[c * bpc:(c + 1) * bpc],
                "m": mf[c * bpc:(c + 1) * bpc],
            }
            for c in range(ncores)
        ]

    trace = os.environ.get("ATTN_TRACE", "0") == "1"
    kw = {}
    if trace and os.environ.get("ATTN_TRACE_DIR"):
        kw["tmpdir"] = os.environ["ATTN_TRACE_DIR"]
    res = run_bass_kernel_spmd(nc, in_maps, list(range(ncores)), trace=trace, **kw)
    if trace:
        LAST_EXEC_TIME_NS = res.exec_time_ns
        LAST_PROFILE = res.profile_json
        globals()["LAST_RESULTS"] = res

    out = np.concatenate([res.results[c]["o"] for c in range(ncores)], axis=0)
    return out
